# revision 1
# baseline (speedup 1.0000x reference)
"""GNN message-passing (gather + segment-sum) Trainium2 kernel.

Strategy (dst-owner sharding, no collectives):
  - Core c owns output nodes [c*NPC, (c+1)*NPC).
  - Host counting-sorts edges into (core, bucket-group, src-block, bucket)
    sub-lists, pads each (bucket, src-block) sub-list to a multiple of 128
    edges (identical padded layout on every core -> one SPMD program).
  - Device per core:
      dma_gather x[src] rows (int16 block-local indices, 4 blocks of 25000
      rows) -> SBUF staging [128 edges, chunk, 64];
      one-hot of bucket-local dst via DVE is_equal against an iota row;
      PE matmul  psum[64 feats, 128 nodes] += msgs^T @ onehot  accumulated
      over a bucket's chunks; ACT copies psum -> SBUF out staging;
      one DMA of [64, NB*128] partial to HBM.
  - Host concatenates the 8 [64, 12500] shards -> [100000, 64].
"""

import sys

for _p in ("/opt/trn_rl_repo", "/root/.axon_site/_ro/trn_rl_repo"):
    if _p not in sys.path:
        sys.path.append(_p)

import numpy as np

from concourse import bass, mybir, tile, bacc
from concourse.bass_utils import run_bass_kernel_spmd

P = 128


def full_cfg():
    return dict(N=100000, D=64, E=1200000, CORES=8, BLOCK_ROWS=25000, GROUP=4,
                NSWQ=4, GMAX=7, STAG_BUFS=6, OH_BUFS=4, SRC_SORT=1, QUANT=64)


def make_layout(edge_index, cfg):
    """Counting-sort edges into the padded SPMD layout.

    Returns (Cmat, meta, per-core arrays).
    """
    N, CORES, BLOCK_ROWS, GROUP = cfg["N"], cfg["CORES"], cfg["BLOCK_ROWS"], cfg["GROUP"]
    NPC = N // CORES
    NB = -(-NPC // P)                       # buckets per core
    NBLK = -(-N // BLOCK_ROWS)              # src blocks
    NG = -(-NB // GROUP)                    # bucket groups

    src = np.asarray(edge_index[0], dtype=np.int64)
    dst = np.asarray(edge_index[1], dtype=np.int64)
    E = src.shape[0]

    core = dst // NPC
    dstl = dst - core * NPC
    bucket = dstl >> 7
    din = (dstl & 127).astype(np.float32)
    order = None
    if cfg.get("BALANCE"):
        # permute each core's buckets by size so slot k holds similar-sized
        # buckets on every core (shrinks the max-over-cores padding term)
        tot = np.bincount(core * NB + bucket, minlength=CORES * NB
                          ).reshape(CORES, NB)
        order = np.argsort(-tot, axis=1, kind="stable")      # slot -> bucket
        slot_of = np.empty_like(order)
        for c in range(CORES):
            slot_of[c, order[c]] = np.arange(NB)
        bucket = slot_of[core, bucket]                       # now slot index
    blk = src // BLOCK_ROWS
    srcl = (src - blk * BLOCK_ROWS).astype(np.int16)
    g = bucket // GROUP
    bing = bucket - g * GROUP

    # per-(core,bucket,blk) counts -> shared padded chunk counts
    cid = (core * NB + bucket) * NBLK + blk
    n = np.bincount(cid, minlength=CORES * NB * NBLK).reshape(CORES, NB, NBLK)
    Cmat = -(-n.max(axis=0) // P)           # [NB, NBLK] chunks
    Cmat[:, 0] = np.maximum(Cmat[:, 0], 1)  # every bucket gets >=1 chunk

    # sub-list start slots in layout order [g][blk][b in g]
    # sub-lists padded to multiples of QUANT (<=128); each (g,blk) unit
    # padded to a multiple of 128 so the chunk grid stays aligned.
    Q = int(cfg.get("QUANT", P))
    assert P % Q == 0
    S = np.zeros((NB, NBLK), dtype=np.int64)        # padded slots per sub-list
    nmax = n.max(axis=0)
    for b in range(NB):
        for bi in range(NBLK):
            m = int(nmax[b, bi])
            if bi == 0:
                m = max(m, 1)
            S[b, bi] = -(-m // Q) * Q
    sub_start = np.zeros((NB, NBLK), dtype=np.int64)
    units = {}                                       # (gi,bi) -> (t0, nck)
    pos = 0
    for gi in range(NG):
        bks = list(range(gi * GROUP, min((gi + 1) * GROUP, NB)))
        for bi in range(NBLK):
            u0 = pos
            for b in bks:
                sub_start[b, bi] = pos
                pos += int(S[b, bi])
            pos = -(-pos // P) * P                   # unit tail pad to x128
            units[(gi, bi)] = (u0 // P, (pos - u0) // P)
    T = pos // P                            # total chunks per core

    # slot -> owning bucket (-1 = pad-tail of a unit); includes sub-list pads
    owner = np.full(T * P, -1, dtype=np.int64)
    for b in range(NB):
        for bi in range(NBLK):
            owner[sub_start[b, bi]:sub_start[b, bi] + S[b, bi]] = b

    # per-unit run table: for each chunk, partition-runs of one bucket
    first_q = {}
    last_q = {}
    for b in range(NB):
        w = np.flatnonzero(owner == b)
        first_q[b], last_q[b] = int(w[0]), int(w[-1])
    runs = {}
    for (gi, bi), (t0, nck) in units.items():
        lst = []
        for tl in range(nck):
            base = (t0 + tl) * P
            rr = []
            j = 0
            while j < P:
                b = int(owner[base + j])
                k = j
                while k < P and int(owner[base + k]) == b:
                    k += Q
                if b >= 0:
                    st = first_q[b] >= base + j and first_q[b] < base + k
                    sp = last_q[b] >= base + j and last_q[b] < base + k
                    # decompose into PE-tile-aligned blocks (128/64/32)
                    blocks = []
                    jj = j
                    while jj < k:
                        for bs in (128, 64, 32):
                            if jj % bs == 0 and jj + bs <= k:
                                blocks.append((jj, bs))
                                jj += bs
                                break
                    for z, (bq, bl) in enumerate(blocks):
                        rr.append((b, bq, bl,
                                   st and z == 0,
                                   sp and z == len(blocks) - 1))
                j = k
            lst.append(rr)
        runs[(gi, bi)] = lst

    # per-edge slot assignment
    sort_key = ((core * NG + g) * NBLK + blk) * GROUP + bing
    if cfg.get("SRC_SORT"):
        perm = np.lexsort((src, sort_key))
    else:
        perm = np.argsort(sort_key, kind="stable")
    rid = sort_key[perm]
    starts = np.r_[0, np.flatnonzero(np.diff(rid)) + 1]
    counts = np.diff(np.r_[starts, E])
    rank = np.arange(E, dtype=np.int64) - np.repeat(starts, counts)
    slot = sub_start[bucket[perm], blk[perm]] + rank
    core_p = core[perm]

    if cfg.get("PADSKIP"):
        src_arr = np.full((CORES, T * P), -1, dtype=np.int16)
    else:
        src_arr = np.zeros((CORES, T * P), dtype=np.int16)
    if cfg.get("ZERO_SRC"):
        srcl[:] = 0
    dst_arr = np.full((CORES, T * P), -1.0, dtype=np.float32)
    src_arr[core_p, slot] = srcl[perm]
    dst_arr[core_p, slot] = din[perm]

    # per-(bucket,blk) call table with per-core valid counts
    calls = []          # (bucket, blk, t0_chunks, n_chunks)
    vcnt = None
    if cfg.get("PADSKIP"):
        for gi in range(NG):
            bks = range(gi * GROUP, min((gi + 1) * GROUP, NB))
            for bi in range(NBLK):
                for b in bks:
                    if Cmat[b, bi] > 0:
                        calls.append((b, bi, int(sub_start[b, bi]) // P, int(Cmat[b, bi])))
        vcnt = np.zeros((CORES, len(calls)), dtype=np.int32)
        for ci, (b, bi, t0, nchk) in enumerate(calls):
            vcnt[:, ci] = np.maximum(n[:, b, bi], 1)
        # calls with zero real edges on a core: make first pad valid (src 0)
        for ci, (b, bi, t0, nchk) in enumerate(calls):
            empty = n[:, b, bi] == 0
            if empty.any():
                src_arr[empty, t0 * P] = 0

    idx_np = np.empty((CORES, P, T * 8), dtype=np.int16)
    dstv_np = np.empty((CORES, P, T), dtype=np.float32)
    for c in range(CORES):
        w = src_arr[c].reshape(T * 8, 16).T          # [16, 8T]
        idx_np[c] = np.tile(w, (8, 1))
        dstv_np[c] = dst_arr[c].reshape(T, P).T      # [128, T]

    meta = dict(NPC=NPC, NB=NB, NBLK=NBLK, NG=NG, T=T, sub_start=sub_start,
                units=units, runs=runs, order=order,
                calls=calls if cfg.get("PADSKIP") else None)
    extras = {"vcnt": vcnt}
    meta["extras"] = extras
    return Cmat, meta, idx_np, dstv_np


def build_nc(Cmat, meta, cfg):
    N, D, CORES, BLOCK_ROWS, GROUP = (
        cfg["N"], cfg["D"], cfg["CORES"], cfg["BLOCK_ROWS"], cfg["GROUP"])
    NB, NBLK, NG, T = meta["NB"], meta["NBLK"], meta["NG"], meta["T"]
    sub_start = meta["sub_start"]
    f32 = mybir.dt.float32

    units, runs = meta["units"], meta["runs"]

    _gq = [0]
    nc = bacc.Bacc(
        None,
        target_bir_lowering=False,
        dynamic_dma_scratch_size=cfg.get("SCRATCH", 16384),
        num_swdge_queues=cfg.get("NSWQ", 1),
    )
    x = nc.dram_tensor("x", [N, D], f32, kind="ExternalInput")
    idx_in = nc.dram_tensor("idx", [P, T * 8], mybir.dt.int16, kind="ExternalInput")
    dstv_in = nc.dram_tensor("dstv", [P, T], f32, kind="ExternalInput")
    iota_in = nc.dram_tensor("iota", [P, P], f32, kind="ExternalInput")
    out = nc.dram_tensor("out", [D, NB * P], f32, kind="ExternalOutput")

    with tile.TileContext(nc) as tc:
        with (
            tc.tile_pool(name="persist", bufs=1) as persist,
            tc.tile_pool(name="stag", bufs=cfg.get("STAG_BUFS", 3)) as stagp,
            tc.tile_pool(name="oh", bufs=cfg.get("OH_BUFS", 2)) as ohp,
            tc.tile_pool(name="psum", bufs=8, space="PSUM") as psump,
        ):
            gsems = [nc.alloc_semaphore(f"gsem{q}") for q in range(cfg.get("NSWQ", 1))] if cfg.get("PREP") else None
            calls = meta.get("calls")
            PADSKIP = cfg.get("PADSKIP") and calls is not None
            if PADSKIP:
                ncalls = len(calls)
                CBMAX = max(c[3] for c in calls)
                vcnt_in = nc.dram_tensor("vcnt", [1, ncalls], mybir.dt.int32, kind="ExternalInput")
                vcnt_t = persist.tile([1, ncalls], mybir.dt.int32)
                nc.sync.dma_start(vcnt_t[:], vcnt_in[:])
                vregs = [nc.gpsimd.alloc_register(name=f"vr{i}") for i in range(4)]
                # map (bucket, blk) -> call index
                call_idx = {(b, bi): ci for ci, (b, bi, _, _) in enumerate(calls)}
                call_tiles = {}
            idx_t = persist.tile([P, T * 8], mybir.dt.int16)
            dstv_t = persist.tile([P, T], f32)
            iota_t = persist.tile([P, P], f32)
            outst = persist.tile([D, NB * P], f32)
            nc.sync.dma_start(idx_t[:], idx_in[:])
            nc.sync.dma_start(dstv_t[:], dstv_in[:])
            nc.sync.dma_start(iota_t[:], iota_in[:])

            if PADSKIP:
                NSLOT = cfg.get("STAG_BUFS", 3)
                stag_all = persist.tile([P, NSLOT, CBMAX, D], f32)
                nc.vector.memset(stag_all[:], 0.0)
            import contextlib
            reps = cfg.get("REPS", 0)
            loop_cm = tc.For_i(0, reps, 1) if reps else contextlib.nullcontext()
            with loop_cm:
              for gi in range(NG):
                  bks = list(range(gi * GROUP, min((gi + 1) * GROUP, NB)))
                  # one psum tile (= one bank) per bucket
                  ptiles = []
                  if not cfg.get("SKIP_COMPUTE"):
                    for h in range(len(bks)):
                      pt_tile = psump.tile([D, P], f32, tag="ps", name=f"ps_{gi}_{h}")
                      ptiles.append(pt_tile)

                  for bi in range(NBLK):
                      t0, nck = units[(gi, bi)]
                      if nck == 0:
                          continue
                      if PADSKIP:
                          NSWQ = cfg.get("NSWQ", 1)
                          # one gather call per (bucket, blk), runtime count reg
                          for b in bks:
                              C_b = int(Cmat[b, bi])
                              if C_b == 0:
                                  continue
                              ci = call_idx[(b, bi)]
                              if ci % 4 == 0:
                                  hi = min(ci + 4, ncalls)
                                  nc.gpsimd.reg_load(vregs[:hi - ci], vcnt_t[0:1, ci:hi])
                              tb = int(sub_start[b, bi]) // P
                              sg = stag_all[:, ci % NSLOT]
                              call_tiles[(b, bi)] = sg
                              q = _gq[0] % NSWQ
                              nc.gpsimd.dma_gather(
                                  sg[:, :C_b, :],
                                  x[bi * BLOCK_ROWS:(bi + 1) * BLOCK_ROWS, :],
                                  idx_t[:, tb * 8:(tb + C_b) * 8],
                                  C_b * P,
                                  vregs[ci % 4],
                                  D,
                                  queue_num=q,
                              )
                              _gq[0] += 1
                          stag = None
                      else:
                          stag = stagp.tile([P, nck, D], f32, tag="st")
                      GMAX = cfg.get("GMAX", 8)
                      NSWQ = cfg.get("NSWQ", 1)
                      if cfg.get("SKIP_GATHER"):
                          nc.gpsimd.memset(stag[:], 0.0)
                      if (not cfg.get("SKIP_GATHER")) and not PADSKIP:
                        for o in range(0, nck, GMAX):
                          w = min(GMAX, nck - o)
                          q = _gq[0] % NSWQ
                          nc.gpsimd.dma_gather(
                              stag[:, o:o + w, :],
                              x[bi * BLOCK_ROWS:(bi + 1) * BLOCK_ROWS, :],
                              idx_t[:, (t0 + o) * 8:(t0 + o + w) * 8],
                              w * P,
                              w * P,
                              D,
                              queue_num=q,
                              single_packet=cfg.get("SINGLE_PACKET", True),
                          )
                          _gq[0] += 1
                      if cfg.get("SKIP_COMPUTE"):
                          continue
                      oh = ohp.tile([P, nck, P], f32, tag="oh")
                      nc.vector.tensor_tensor(
                          out=oh[:],
                          in0=dstv_t[:, t0:t0 + nck].to_broadcast([P, nck, P]),
                          in1=iota_t[:, None, :].to_broadcast([P, nck, P]),
                          op=mybir.AluOpType.is_equal,
                      )
                      for tl in range(nck):
                          for (b, qpos, nq, st, sp) in runs[(gi, bi)][tl]:
                              pt = ptiles[b - bks[0]]
                              kw = {}
                              if qpos > 0:
                                  kw["tile_position"] = (qpos, 0)
                              nc.tensor.matmul(
                                  out=pt[:, :],
                                  lhsT=stag[qpos:qpos + nq, tl, :],
                                  rhs=oh[qpos:qpos + nq, tl, :],
                                  start=st,
                                  stop=sp,
                                  **kw,
                              )

                  for h, pt in enumerate(ptiles):
                      c0 = (bks[0] + h) * P
                      nc.scalar.copy(out=outst[:, c0:c0 + P], in_=pt[:, :])
                  if cfg.get("SKIP_COMPUTE") and gi == 0:
                      nc.vector.memset(outst[:], 0.0)
                  if cfg.get("OUTSPLIT", 1):
                      g0 = bks[0] * P
                      g1 = (bks[-1] + 1) * P
                      nc.sync.dma_start(out[:, g0:g1], outst[:, g0:g1])

            if not cfg.get("OUTSPLIT", 1):
                nc.sync.dma_start(out[:], outst[:])
    nc.finalize()
    return nc


_CACHE = {}


def _get_nc(Cmat, meta, cfg):
    key = (meta["sub_start"].tobytes(), meta["T"], cfg["N"], cfg["D"],
           cfg["CORES"], cfg.get("QUANT", P))
    if key not in _CACHE:
        _CACHE[key] = build_nc(Cmat, meta, cfg)
    return _CACHE[key]


def make_in_maps(x, idx_np, dstv_np, cfg, meta=None):
    CORES, D = cfg["CORES"], cfg["D"]
    xf = np.ascontiguousarray(np.asarray(x, dtype=np.float32))
    iota = np.broadcast_to(np.arange(P, dtype=np.float32), (P, P)).copy()
    maps = [
        {"x": xf, "idx": idx_np[c], "dstv": dstv_np[c], "iota": iota}
        for c in range(CORES)
    ]
    if meta is not None and meta.get("extras", {}).get("vcnt") is not None:
        vc = meta["extras"]["vcnt"]
        for c in range(CORES):
            maps[c]["vcnt"] = vc[c:c + 1]
    return maps


def assemble(shards, meta, cfg):
    N, D, CORES = cfg["N"], cfg["D"], cfg["CORES"]
    NPC, NB = meta["NPC"], meta["NB"]
    order = meta.get("order")
    if order is None:
        full = np.concatenate([sh[:, :NPC] for sh in shards], axis=1).T
        return np.ascontiguousarray(full)
    full = np.empty((N, D), dtype=np.float32)
    for c in range(CORES):
        for k in range(NB):
            gb = int(order[c][k])
            r0 = c * NPC + gb * P
            r1 = min(r0 + P, (c + 1) * NPC)
            if r0 >= r1:
                continue
            full[r0:r1] = shards[c][:, k * P:k * P + (r1 - r0)].T
    return full


def kernel(x, edge_index):
    cfg = full_cfg()
    Cmat, meta, idx_np, dstv_np = make_layout(edge_index, cfg)
    nc = _get_nc(Cmat, meta, cfg)
    in_maps = make_in_maps(x, idx_np, dstv_np, cfg, meta)
    res = run_bass_kernel_spmd(nc, in_maps, core_ids=list(range(cfg["CORES"])))
    shards = [res.results[c]["out"] for c in range(cfg["CORES"])]
    return assemble(shards, meta, cfg)



# revision 10
# speedup vs baseline: 1.3899x; 1.3899x over previous
"""GNN message-passing (gather + segment-sum) Trainium2 kernel.

Strategy (dst-owner sharding, no collectives), v2:
  - Core c owns output nodes [c*NPC, (c+1)*NPC).
  - x is converted to bf16 on HOST and packed as xb[50000, 128]: row k holds
    nodes 2k (cols 0:64) and 2k+1 (cols 64:128).  Device gathers 128B rows at
    256B HBM stride (half the per-descriptor DMA cost of 256B f32 rows).
    A gather call's HBM base picks (block, parity): block = pair-index>>15
    (int16 index limit), parity = src&1 selects col 0:64 / 64:128.
  - Host counting-sorts edges into (core, group, class, bucket) sub-lists,
    padded to QUANT multiples with identical layout on every core (SPMD).
  - Device per core, for each group of GROUP buckets:
      one dma_gather per (group, class) -> stag [128 slots, nck, 64] bf16;
      one-hot of bucket-local dst via DVE is_equal, built COLUMN-MAJOR
      ([part, dstcol, chunk]) so every operand has a packed 2-byte last dim
      -> DVE 2x mode;
      PE matmuls emitted BUCKET-major (each bucket's chunks across all 4
      classes consecutively): psum[128 dst, 64 feat] += onehot^T @ msgs.
      PSUM accumulation groups must be sequential within a bank - regions
      of one bank cannot have interleaved open groups (HW-verified).
      ACT copies psum[128, G*64] -> SBUF out staging; per-group DMA to HBM.
  - Output HBM layout [128, NB*64]: (p, b*64+f) = node b*128+p, feature f.
    Host reshapes/transposes the 8 shards -> [100000, 64] f32.
"""

import sys

for _p in ("/opt/trn_rl_repo", "/root/.axon_site/_ro/trn_rl_repo"):
    if _p not in sys.path:
        sys.path.append(_p)

import numpy as np

from concourse import bass, mybir, tile, bacc
from concourse._compat import exact_div, round_up_to_multiple
from concourse.bass_primitives import MemorySpace
from concourse.bass_utils import run_bass_kernel_spmd
import concourse.ap_utils as ap_utils

P = 128
NPAIR = 50000          # pair rows in xb
PAIR_BLOCK = 32768     # pair-index block boundary (int16 range)


def full_cfg():
    # QUANT=128: every chunk belongs to exactly one bucket, so no matmul ever
    # needs tile_position.  tile_position with bf16 operands crashes the
    # device (FWL + row-offset conflict; f32 worked in the old kernel).
    return dict(N=100000, D=64, E=1200000, CORES=8, GROUP=8, QUANT=128,
                NSWQ=4, STAG_BUFS=6, OH_BUFS=4, SCRATCH=65536, SRC_SORT=1)


def _dma_gather_small(gp, out_ap, in_ap, idxs_ap, num_idxs, num_idxs_reg,
                      elem_size, elem_step, queue_num=0, single_packet=True):
    """nc.gpsimd.dma_gather minus the elem_size_bytes%256 assert.

    The %256 restriction only applies to the transpose path (see
    decode/dma_gather.hpp); the non-transpose Q7 desc-gen emits one
    descriptor of elem_size_bytes per index for any size.  The HBM stride
    (elem_step) must still be a multiple of 256B.
    """
    gp._assert_queue_num(queue_num)
    assert idxs_ap.dtype == mybir.dt.int16
    assert in_ap.dtype == out_ap.dtype
    elem_size_bytes = elem_size * mybir.dt.size(in_ap.dtype)
    assert elem_size_bytes > 0 and elem_size_bytes % 4 == 0
    assert in_ap.space == MemorySpace.DRAM
    assert idxs_ap.space == MemorySpace.SBUF
    assert out_ap.space == MemorySpace.SBUF
    assert ap_utils.ap_is_contiguous(out_ap.ap[1:])
    assert ap_utils.ap_is_contiguous(idxs_ap.ap[1:])
    assert out_ap.ap[0][1] * out_ap.ap[1][1] == round_up_to_multiple(num_idxs, 128)
    assert in_ap.ap[-1][1] == out_ap.ap[-1][1] == elem_size
    assert in_ap.ap[0][0] == elem_step
    stride_bytes = elem_step * mybir.dt.size(in_ap.dtype)
    stride_bytes_256 = exact_div(stride_bytes, 256)
    assert stride_bytes_256 < 256

    _in_ap = gp.lower_ap_dma(in_ap, for_custom_bir_dma=True)
    _idxs_ap = gp.lower_ap(idxs_ap)
    _out_ap = gp.lower_ap(out_ap)
    return gp.add_instruction(
        mybir.InstDMAGatherAnt(
            name=gp.bass.get_next_instruction_name(),
            ins=[
                *_in_ap,
                _idxs_ap,
                gp.lower_val_access(gp.to_reg(num_idxs_reg)),
            ],
            outs=[_out_ap],
            transpose=False,
            num_idxs=num_idxs,
            elem_size=elem_size,
            stride_bytes_256=stride_bytes_256,
            gen_mode=0,
            single_packet=single_packet,
            queue_num=queue_num,
            sbuf_tokens_per_rank=0,
            sbuf_free_dim_per_rank=0,
            sbuf_free_dim_pad_per_rank=0,
            sbuf_byte_offset=0,
        )
    )


def make_layout(edge_index, cfg):
    """Counting-sort edges into the padded SPMD (group, class, bucket) layout."""
    N, CORES, Q, G = cfg["N"], cfg["CORES"], cfg["QUANT"], cfg["GROUP"]
    NPC = N // CORES                      # 12500
    NB = -(-NPC // P)                     # 98 buckets per core
    NCLS = 4
    NG = -(-NB // G)

    src = np.asarray(edge_index[0], dtype=np.int64)
    dst = np.asarray(edge_index[1], dtype=np.int64)
    E = src.shape[0]

    core = dst // NPC
    dstl = dst - core * NPC
    bucket = dstl >> 7
    din = (dstl & 127).astype(np.float32)
    pr = (src & 1)
    half = src >> 1
    blk = (half >= PAIR_BLOCK).astype(np.int64)
    lidx = (half - blk * PAIR_BLOCK).astype(np.int16)
    cls = blk * 2 + pr

    # per-(core, bucket, class) counts -> shared padded sub-list sizes
    cid = (core * NB + bucket) * NCLS + cls
    n = np.bincount(cid, minlength=CORES * NB * NCLS).reshape(CORES, NB, NCLS)
    nmax = n.max(axis=0)
    S = (-(-nmax // Q) * Q).astype(np.int64)      # [NB, NCLS] slots per sub-list
    S[:, 0] = np.maximum(S[:, 0], Q)              # every bucket gets >=1 run

    # layout order: [group][class][bucket in group]; units padded to x128
    sub_start = np.zeros((NB, NCLS), dtype=np.int64)
    units = {}                                    # (gi, c) -> (chunk0, nchunks)
    pos = 0
    for gi in range(NG):
        bks = list(range(gi * G, min((gi + 1) * G, NB)))
        for c in range(NCLS):
            u0 = pos
            for b in bks:
                sub_start[b, c] = pos
                pos += int(S[b, c])
            pos = -(-pos // P) * P
            units[(gi, c)] = (u0 // P, (pos - u0) // P)
    T = pos // P
    nckmax = max(nck for (_, nck) in units.values())

    # owner bookkeeping for pads
    owner = np.full(T * P, -1, dtype=np.int64)
    for b in range(NB):
        for c in range(NCLS):
            if S[b, c]:
                owner[sub_start[b, c]:sub_start[b, c] + S[b, c]] = b

    # per-group, BUCKET-major matmul schedule.  PSUM accumulation groups must
    # be sequential: all of bucket b's matmuls (across its 4 class sub-lists)
    # are emitted consecutively, start on the first, stop on the last.
    # Entry: (b, [(c, tl, qpos, nq), ...]) with tl local to unit (gi, c).
    sched = {}
    for gi in range(NG):
        bks = list(range(gi * G, min((gi + 1) * G, NB)))
        glist = []
        for b in bks:
            ent = []
            for c in range(NCLS):
                s0, ln = int(sub_start[b, c]), int(S[b, c])
                if ln == 0:
                    continue
                t0, _ = units[(gi, c)]
                j = s0
                while j < s0 + ln:
                    tl = j // P - t0
                    qpos = j % P
                    k = min(s0 + ln, (j // P + 1) * P)   # end within this chunk
                    # decompose [qpos, k-local) into 128/64/32 blocks
                    jj = qpos
                    kk = k - (j - qpos)                  # local end
                    while jj < kk:
                        for bs in (128, 64, 32):
                            if jj % bs == 0 and jj + bs <= kk:
                                ent.append((c, tl, jj, bs))
                                jj += bs
                                break
                    j = k
            glist.append((b, ent))
        sched[gi] = glist

    # per-edge slot assignment
    key = (core * NB + bucket) * NCLS + cls
    if cfg.get("SRC_SORT"):
        perm = np.lexsort((src, key))
    else:
        perm = np.argsort(key, kind="stable")
    rid = key[perm]
    starts = np.r_[0, np.flatnonzero(np.diff(rid)) + 1]
    counts = np.diff(np.r_[starts, E])
    rank = np.arange(E, dtype=np.int64) - np.repeat(starts, counts)
    slot = sub_start[bucket[perm], cls[perm]] + rank
    core_p = core[perm]

    src_arr = np.full((CORES, T * P), -1, dtype=np.int16)
    src_arr[:, owner >= 0] = 0                    # in-sublist pads gather row 0
    dst_arr = np.full((CORES, T * P), -1.0, dtype=np.float32)
    src_arr[core_p, slot] = lidx[perm]
    dst_arr[core_p, slot] = din[perm]

    bf16 = mybir.dt.np(mybir.dt.bfloat16)
    idx_np = np.empty((CORES, P, T * 8), dtype=np.int16)
    dstv_np = np.empty((CORES, P, T), dtype=bf16)
    for c in range(CORES):
        w = src_arr[c].reshape(T * 8, 16).T       # [16, 8T]
        idx_np[c] = np.tile(w, (8, 1))
        dstv_np[c] = dst_arr[c].reshape(T, P).T.astype(bf16)

    meta = dict(NPC=NPC, NB=NB, NG=NG, T=T, sub_start=sub_start, S=S,
                units=units, sched=sched, nckmax=nckmax)
    return S, meta, idx_np, dstv_np


def build_nc(S, meta, cfg):
    N, D, CORES, G = cfg["N"], cfg["D"], cfg["CORES"], cfg["GROUP"]
    NB, NG, T = meta["NB"], meta["NG"], meta["T"]
    units, sched, NCKMAX = meta["units"], meta["sched"], meta["nckmax"]
    f32 = mybir.dt.float32
    bf16 = mybir.dt.bfloat16
    NSWQ = cfg.get("NSWQ", 4)

    _gq = [0]
    nc = bacc.Bacc(
        None,
        target_bir_lowering=False,
        dynamic_dma_scratch_size=cfg.get("SCRATCH", 65536),
        num_swdge_queues=NSWQ,
    )
    xb = nc.dram_tensor("xb", [NPAIR, 2 * D], bf16, kind="ExternalInput")
    idx_in = nc.dram_tensor("idx", [P, T * 8], mybir.dt.int16, kind="ExternalInput")
    dstv_in = nc.dram_tensor("dstv", [P, T], bf16, kind="ExternalInput")
    iota_in = nc.dram_tensor("iota", [P, P, NCKMAX], bf16, kind="ExternalInput")
    out = nc.dram_tensor("out", [P, NB * D], f32, kind="ExternalOutput")

    with tile.TileContext(nc) as tc:
        with (
            tc.tile_pool(name="persist", bufs=1) as persist,
            tc.tile_pool(name="stag", bufs=cfg.get("STAG_BUFS", 8)) as stagp,
            tc.tile_pool(name="oh", bufs=cfg.get("OH_BUFS", 6)) as ohp,
            tc.tile_pool(name="psum", bufs=8, space="PSUM") as psump,
        ):
            idx_t = persist.tile([P, T * 8], mybir.dt.int16)
            dstv_t = persist.tile([P, T], bf16)
            iota_t = persist.tile([P, P, NCKMAX], bf16)
            outst = persist.tile([P, NB * D], f32)
            nc.sync.dma_start(idx_t[:], idx_in[:])
            nc.sync.dma_start(dstv_t[:], dstv_in[:])
            nc.sync.dma_start(iota_t[:, :, :], iota_in[:, :, :])

            import contextlib
            reps = cfg.get("REPS", 0)
            loop_cm = tc.For_i(0, reps, 1) if reps else contextlib.nullcontext()
            with loop_cm:
                for gi in range(NG):
                    bks = list(range(gi * G, min((gi + 1) * G, NB)))
                    GA = len(bks)
                    pt = psump.tile([P, G * D], f32, tag="ps", name=f"ps_{gi}")
                    stags, ohs = {}, {}
                    for c in range(4):
                        t0, nck = units[(gi, c)]
                        if nck == 0:
                            continue
                        stag = stagp.tile([P, nck, D], bf16, tag="st")
                        stags[c] = stag
                        blk, par = c >> 1, c & 1
                        r0 = blk * PAIR_BLOCK
                        r1 = NPAIR if blk else PAIR_BLOCK
                        q = _gq[0] % NSWQ
                        _gq[0] += 1
                        if cfg.get("SKIP_GATHER"):
                            nc.gpsimd.memset(stag[:], 0.0)
                        else:
                            _dma_gather_small(
                                nc.gpsimd,
                                stag[:, :, :],
                                xb[r0:r1, par * D:(par + 1) * D],
                                idx_t[:, t0 * 8:(t0 + nck) * 8],
                                nck * P,
                                nck * P,
                                D,
                                2 * D,
                                queue_num=q,
                                # single_packet coalesces a call's descs per
                                # DMA engine into ONE SDMA packet, capped at
                                # 16KB: (nck*128/16+1)*128B overflows for
                                # nck>15 -> device crash.  Disable.
                                single_packet=False,
                            )
                        # column-major one-hot [part, dstcol, chunk]: every
                        # operand has a packed 2-byte last dim -> DVE 2x mode
                        oh = ohp.tile([P, P, nck], bf16, tag="oh")
                        ohs[c] = oh
                        if cfg.get("SKIP_OH"):
                            nc.vector.memset(oh[:], 0.0)
                        else:
                            nc.vector.tensor_tensor(
                                out=oh[:],
                                in0=dstv_t[:, None, t0:t0 + nck].to_broadcast(
                                    [P, P, nck]),
                                in1=iota_t[:, :, :nck],
                                op=mybir.AluOpType.is_equal,
                            )
                    if cfg.get("SKIP_MM"):
                        if gi == 0:
                            nc.vector.memset(outst[:], 0.0)
                        nc.sync.dma_start(out[:, bks[0] * D:(bks[-1] + 1) * D],
                                          outst[:, bks[0] * D:(bks[-1] + 1) * D])
                        continue
                    for (b, ent) in sched[gi]:
                        h = b - bks[0]
                        for z, (c, tl, qpos, nq) in enumerate(ent):
                            kw = {}
                            if qpos > 0:
                                kw["tile_position"] = (qpos, 0)
                            nc.tensor.matmul(
                                out=pt[:, h * D:(h + 1) * D],
                                lhsT=ohs[c][qpos:qpos + nq, :, tl],
                                rhs=stags[c][qpos:qpos + nq, tl, :],
                                start=(z == 0),
                                stop=(z == len(ent) - 1),
                                **kw,
                            )
                    c0 = bks[0] * D
                    c1 = (bks[-1] + 1) * D
                    nc.scalar.copy(out=outst[:, c0:c1], in_=pt[:, :GA * D])
                    nc.sync.dma_start(out[:, c0:c1], outst[:, c0:c1])
    nc.finalize()
    return nc


_CACHE = {}


def _get_nc(S, meta, cfg):
    key = (meta["sub_start"].tobytes(), meta["T"], cfg["N"], cfg["D"],
           cfg["CORES"], cfg["GROUP"], cfg["QUANT"])
    if key not in _CACHE:
        _CACHE[key] = build_nc(S, meta, cfg)
    return _CACHE[key]


def make_in_maps(x, idx_np, dstv_np, cfg, meta):
    CORES = cfg["CORES"]
    bf16 = mybir.dt.np(mybir.dt.bfloat16)
    xb = np.ascontiguousarray(
        np.asarray(x, dtype=np.float32).astype(bf16).reshape(NPAIR, 2 * cfg["D"])
    )
    iota = np.ascontiguousarray(
        np.broadcast_to(
            np.arange(P, dtype=np.float32)[None, :, None],
            (P, P, meta["nckmax"]),
        ).astype(bf16)
    )
    return [
        {"xb": xb, "idx": idx_np[c], "dstv": dstv_np[c], "iota": iota}
        for c in range(CORES)
    ]


def assemble(shards, meta, cfg):
    N, D, CORES = cfg["N"], cfg["D"], cfg["CORES"]
    NPC, NB = meta["NPC"], meta["NB"]
    full = np.empty((N, D), dtype=np.float32)
    for c in range(CORES):
        arr = shards[c].reshape(P, NB, D).transpose(1, 0, 2).reshape(NB * P, D)
        full[c * NPC:(c + 1) * NPC] = arr[:NPC]
    return full


def kernel(x, edge_index):
    cfg = full_cfg()
    S, meta, idx_np, dstv_np = make_layout(edge_index, cfg)
    nc = _get_nc(S, meta, cfg)
    in_maps = make_in_maps(x, idx_np, dstv_np, cfg, meta)
    res = run_bass_kernel_spmd(nc, in_maps, core_ids=list(range(cfg["CORES"])))
    shards = [res.results[c]["out"] for c in range(cfg["CORES"])]
    return assemble(shards, meta, cfg)
